# revision 37
# baseline (speedup 1.0000x reference)
"""RNN-T joint network kernel for Trainium2 (Bass/Tile), 8-core data-parallel.

Problem: out[b,t,u,:] = tanh(enc[b,t]@W_enc + b_enc + dec[b,u]@W_dec + b_dec) @ W_out + b_out
Shapes: B=8, T=256, U=64, D=512, J=640, V=1024 (all fp32).

Sharding: data-parallel over batch B across the 8 NeuronCores (1 batch element
per core). Per core the dominant work is the joint matmul (T,U,J)x(J,V):
1280 bf16 matmuls of N=512 -> ~276us at the 2.4 GHz PE clock. The 64MB fp32
output DMA (~187us at 358 GB/s) and the tanh/drain engines fit underneath.

Changes vs the 304us baseline (whose head was ~24us: serial const DMAs +
cold-clock (HAM K=4/8) projections; and whose fp32 output stream
rate-matched the ~250GB/s DMA envelope, backlogging the queues):
  - fp16 output (adds ~5e-4 rel err; total 3.4e-3 vs the 2e-2 gate), host
    upcasts to fp32 during the gather. Halves output traffic.
  - const cargo in medium (~256-384KB) need-ordered chunks split across
    both HWDGE queues: each dma_start costs ~650ns serial issue time on its
    engine, each DMA's completion sem fires ~1.5-3us after its last byte,
    so medium chunks on two queues pipeline best (~370GB/s in flight).
  - a short stream of dummy matmuls on zeroed SBUF warms the PE HAM clock
    gate during the DMA head, so projections + main loop run at 2.4 GHz.
  - bias_rep built on-device with K=1 PE broadcast matmuls (was a 512KB
    SWDGE broadcast DMA).
  - split-u head: the first U_SPLIT u's emit only their vv0 groups (vv0
    w_out half lands first), the vv1 halves backfill at u in
    [U_SPLIT, 2*U_SPLIT) — PE stays busy through the w_out arrival tail.
  - last-u drains per v-half with immediate small DMAs on the two HWDGE
    queues (SWDGE has ~2us completion latency) to shrink the tail.

Per-core layout (all J-major so J is the matmul contraction partition dim):
  host:   inputs pre-transposed and pre-packed per 128-row chunk; projection
          weights jc-major, W_out vv-major (so chunk DMAs are contiguous).
  setup:  enc_projT[j,t] = W_enc^T @ encT, dec_projT likewise with
          (b_enc+b_dec) folded in via ACT bias on the drain.
  main:   per u: hT[j,t] = tanh(enc_projT[j,t] + dec_projT[j,u]) via ACT;
          per (u,tt): two 5-mm PSUM groups (one per v-half), two [128,512]
          DVE drains adding broadcast b_out into one fp16 stage, one 512KB
          DMA (4KB/partition lines) alternating sync/gpsimd.
"""

import numpy as np
from contextlib import ExitStack

from concourse import bacc, bass, tile
from concourse.bass import mybir
from concourse.bass_utils import run_bass_kernel_spmd

F32 = mybir.dt.float32
F16 = mybir.dt.float16
BF16 = mybir.dt.bfloat16
ACT_F = mybir.ActivationFunctionType

B, T, U = 8, 256, 64
D, J, V = 512, 640, 1024
NJC = J // 128   # 5 contraction chunks of the joint matmul
NDC = D // 128   # 4 contraction chunks of the projections
NVB = V // 512   # 2 v-halves (one psum bank each) per joint output tile
N_WARM = 8       # dummy matmuls to warm the HAM clock gate during DMA head
U_SPLIT = 4      # leading u's run vv0-only, their vv1 halves backfill later
NEA = 2          # w_enc jc-chunks in the first (early) DMA tile


def build_program() -> bass.Bass:
    nc = bacc.Bacc("TRN2", target_bir_lowering=False, debug=False)

    # packed layouts (see _make_in_maps): projections jc-major, w_out vv-major
    encT_d = nc.declare_dram_parameter("encT", [128, NDC * T], BF16, isOutput=False)
    decT_d = nc.declare_dram_parameter("decT", [128, NDC * U], BF16, isOutput=False)
    w_enc_d = nc.declare_dram_parameter("w_enc", [128, NJC * D], BF16, isOutput=False)
    w_dec_d = nc.declare_dram_parameter("w_dec", [128, NJC * D], BF16, isOutput=False)
    bb_d = nc.declare_dram_parameter("bb", [128, NJC], F32, isOutput=False)  # b_enc+b_dec
    w_out_d = nc.declare_dram_parameter("w_out", [128, NJC * V], BF16, isOutput=False)
    b_out_d = nc.declare_dram_parameter("b_out", [1, V], BF16, isOutput=False)
    # fp16 output (rel err ~5e-4, negligible vs the bf16 matmul error): halves
    # the 64MB output stream, which otherwise rate-matches the ~250 GB/s
    # per-core DMA envelope and backlogs the queues
    out = nc.declare_dram_parameter("out", [T, U, V], F16, isOutput=True)

    with tile.TileContext(nc) as tc, ExitStack() as ctx:
        const = ctx.enter_context(tc.tile_pool(name="const", bufs=1))

        # --- PE warmup: HAM un-throttles after ~3.4us of sustained matmul
        # activity; run dummies on zeroed SBUF while the const DMAs stream in
        # so the projections and main loop start at 2.4 GHz instead of 1.2.
        warm_sb = const.tile([128, 512], BF16, name="warm")
        nc.vector.memset(warm_sb[:], 0.0)
        ones_sb = const.tile([1, 128], BF16, name="ones")
        nc.vector.memset(ones_sb[:], 1.0)

        # --- resident constants, split across the two HWDGE queues so the
        # dec path (sync) and enc path (scalar) load in parallel ------------
        # The const load is HBM-bandwidth-walled (~265 GB/s effective), so the
        # ORDER is what matters: dec-projection cargo first (sync), enc cargo
        # on the second HWDGE queue (scalar), then w_out chunks progressively
        # (the main loop consumes vv0 first); vv1's first chunks ride the
        # otherwise-idle gpsimd SWDGE queue. w_out chunks are separate tiles
        # so each matmul releases as soon as its own 128KB slice lands.
        # Const cargo tuning (all measured): each dma_start costs ~650ns of
        # serial ISSUE time on its engine; each DMA's completion SEMAPHORE
        # fires ~1.5-3us after its last byte (worse for bigger transfers);
        # the fabric does ~370 GB/s when >=2 transfers are in flight. So:
        # medium (~256-384KB) chunks, split across BOTH HWDGE queues for
        # parallel issue, strictly need-ordered, few enough on the scalar
        # queue that its issues retire before the projection drains + tanh.
        # sync leads with the dproj cargo (decT + w_dec) — b_out/bb are only
        # consumed later (bias matmuls / projection drains) and each DMA's
        # completion sem lags its data by ~2-3us, so first-needed goes first
        decT = const.tile([128, NDC * U], BF16)
        nc.sync.dma_start(out=decT[:], in_=decT_d[:])
        encT = const.tile([128, NDC * T], BF16)
        nc.scalar.dma_start(out=encT[:], in_=encT_d[:])

        def chunked(dram, name, splits, queues):
            """Load [128, NJC*D-like] weight as jc-chunk tiles: splits is a
            list of jc-counts; returns (tiles, per-chunk jc offset map)."""
            tiles = []
            jmap = {}
            jc0 = 0
            width = dram.shape[1] // NJC
            for si, njc in enumerate(splits):
                t = const.tile([128, njc * width], BF16, name=f"{name}{si}")
                queues[si].dma_start(
                    out=t[:], in_=dram[:, jc0 * width : (jc0 + njc) * width]
                )
                for j in range(njc):
                    jmap[jc0 + j] = (t, j)
                jc0 += njc
            return jmap

        w_dec_m = chunked(w_dec_d, "wd", [2, 3], [nc.sync, nc.sync])
        w_enc_m = chunked(w_enc_d, "we", [2, 1, 2], [nc.scalar, nc.scalar, nc.scalar])
        b_out_sb = const.tile([1, V], BF16)
        nc.sync.dma_start(out=b_out_sb[:], in_=b_out_d[:])
        bbt = const.tile([128, NJC], F32)
        nc.sync.dma_start(out=bbt[:], in_=bb_d[:])

        def w_slice(jmap, jc, dc):
            t, j = jmap[jc]
            return t[:, j * D + dc * 128 : j * D + (dc + 1) * 128]

        # w_out: per v-half, two chunk tiles (jc0-2, jc3-4), vv-major layout
        half = NJC * 512
        w_out_m = []
        for vv in range(NVB):
            m = {}
            for si, (j0, njc) in enumerate([(0, 3), (3, 2)]):
                t = const.tile([128, njc * 512], BF16, name=f"wo{vv}{si}")
                nc.sync.dma_start(
                    out=t[:],
                    in_=w_out_d[:, vv * half + j0 * 512 : vv * half + (j0 + njc) * 512],
                )
                for j in range(njc):
                    m[j0 + j] = (t, j)
            w_out_m.append(m)

        bias_rep = const.tile([128, V], F32)
        enc_projT = [const.tile([128, T], F32, name=f"ep{jc}") for jc in range(NJC)]
        dec_projT = [const.tile([128, U], F32, name=f"dp{jc}") for jc in range(NJC)]

        with tc.tile_pool(name="setup_ps", bufs=1, space="PSUM") as setup_ps:
            # dummy matmuls first in program order -> head of the PE queue.
            # MUST alternate 2 banks: same-bank WAW serializes at the
            # isolated ~610ns latency and that duty cycle does NOT trigger
            # the HAM un-throttle — the whole setup then runs at 1.2 GHz.
            for i in range(N_WARM):
                wps = setup_ps.tile([128, 512], F32, tag=f"warm{i % 2}")
                nc.tensor.matmul(
                    wps[:], warm_sb[:, :128], warm_sb[:], start=True, stop=True,
                )

            # bias_rep[p, v] = b_out[v] via K=1 broadcast matmuls; two passes
            # through ONE psum bank (pass 2 is emitted after the dec
            # projections so its WAW wait never blocks them in the PE FIFO)
            bps = setup_ps.tile([128, 512], F32, tag="biasb")
            nc.tensor.matmul(
                bps[:], ones_sb[:], b_out_sb[:, :512], start=True, stop=True,
            )
            nc.vector.tensor_copy(bias_rep[:, :512], bps[:])

            # --- input projections (bf16 mms, fp32 accumulation) ------------
            for jc in range(NJC):
                ps = setup_ps.tile([128, U], F32, tag=f"dproj{jc % 2}")
                for dc in range(NDC):
                    nc.tensor.matmul(
                        ps[:],
                        w_slice(w_dec_m, jc, dc),
                        decT[:, dc * U : (dc + 1) * U],
                        start=(dc == 0),
                        stop=(dc == NDC - 1),
                    )
                # fold b_enc+b_dec into dec_projT during the PSUM->SBUF drain
                nc.scalar.activation(
                    dec_projT[jc][:], ps[:], ACT_F.Identity,
                    bias=bbt[:, jc : jc + 1], scale=1.0,
                )

            bps2 = setup_ps.tile([128, 512], F32, tag="biasb")
            nc.tensor.matmul(
                bps2[:], ones_sb[:], b_out_sb[:, 512:], start=True, stop=True,
            )
            nc.vector.tensor_copy(bias_rep[:, 512:], bps2[:])

            for jc in range(NJC):
                ps = setup_ps.tile([128, T], F32, tag=f"eproj{jc % 3}")
                for dc in range(NDC):
                    nc.tensor.matmul(
                        ps[:],
                        w_slice(w_enc_m, jc, dc),
                        encT[:, dc * T : (dc + 1) * T],
                        start=(dc == 0),
                        stop=(dc == NDC - 1),
                    )
                nc.vector.tensor_copy(enc_projT[jc][:], ps[:])
                if jc == 1:
                    # PE filler: the w_enc/w_dec tail chunks' completion
                    # sems lag their data by ~2us; keep the PE busy through
                    # that window so HAM holds 2.4 GHz for the rest of setup
                    for i in range(8):
                        wps = setup_ps.tile([128, 512], F32, tag=f"warm{i % 2}")
                        nc.tensor.matmul(
                            wps[:], warm_sb[:, :128], warm_sb[:],
                            start=True, stop=True,
                        )

        # --- main loop, one u per iteration ---------------------------------
        # Steady state: per (u,tt) two 5-mm PSUM groups (one per v-half), two
        # [128,512] DVE drains (+ broadcast b_out) into one fp16 stage, one
        # 512KB DMA (4KB/partition lines) alternating sync/gpsimd.
        # Head: the PE consumes w_out chunks ~2x faster than HBM delivers
        # them, so u<U_SPLIT emit only their vv0 groups (vv0 chunks land
        # first) and the vv1 halves backfill at u in [U_SPLIT, 2*U_SPLIT) —
        # the PE stays busy through the w_out arrival tail.
        # Tail: last u drains per v-half with immediate small DMAs on the two
        # HWDGE queues (SWDGE has ~2us completion latency).
        h_pool = ctx.enter_context(tc.tile_pool(name="h", bufs=12))
        st_pool = ctx.enter_context(tc.tile_pool(name="stage", bufs=1))
        mm_ps = ctx.enter_context(tc.tile_pool(name="mm_ps", bufs=1, space="PSUM"))
        NST = 6   # full-stage ring depth
        NSH = 6   # half-stage ring depth

        gi = 0  # psum bank ring counter
        di = 0  # output DMA queue alternation counter
        hT_hist = {}

        def mm_group(hT, tt, vv):
            nonlocal gi
            ps = mm_ps.tile([128, 512], F32, tag=f"mm{gi % 8}", name="ps")
            gi += 1
            for jc in range(NJC):
                wt, wj = w_out_m[vv][jc]
                nc.tensor.matmul(
                    ps[:],
                    hT[jc][:, tt * 128 : (tt + 1) * 128],
                    wt[:, wj * 512 : (wj + 1) * 512],
                    start=(jc == 0),
                    stop=(jc == NJC - 1),
                )
            return ps

        def half_group(hT, u, tt, vv, hwdge_only=False):
            """5-mm group + [128,512] drain + 256KB DMA for one v-half."""
            nonlocal di
            ps = mm_group(hT, tt, vv)
            stage = st_pool.tile([128, 1, 512], F16, tag=f"sh{di % NSH}", name="hstage")
            nc.vector.tensor_add(
                stage[:, 0, :], ps[:], bias_rep[:, vv * 512 : (vv + 1) * 512]
            )
            if hwdge_only:
                q = nc.sync if di % 2 == 0 else nc.scalar
            else:
                q = nc.sync if di % 2 == 0 else nc.gpsimd
            di += 1
            q.dma_start(
                out=out[tt * 128 : (tt + 1) * 128, u : u + 1, vv * 512 : (vv + 1) * 512],
                in_=stage[:],
            )

        for u in range(U):
            hT = [h_pool.tile([128, T], BF16, tag=f"h{jc}", name=f"h{jc}") for jc in range(NJC)]
            hT_hist[u] = hT
            for jc in range(NJC):
                nc.scalar.activation(
                    hT[jc][:],
                    enc_projT[jc][:],
                    ACT_F.Tanh,
                    bias=dec_projT[jc][:, u : u + 1],
                    scale=1.0,
                )
            if U_SPLIT <= u < 2 * U_SPLIT:
                # backfill the deferred vv1 halves of u - U_SPLIT
                ub = u - U_SPLIT
                for tt in range(T // 128):
                    half_group(hT_hist[ub], ub, tt, 1)
            if u < U_SPLIT:
                for tt in range(T // 128):
                    half_group(hT_hist[u], u, tt, 0)
            elif u < U - 1:
                for tt in range(T // 128):
                    psA = mm_group(hT, tt, 0)
                    psB = mm_group(hT, tt, 1)
                    stage = st_pool.tile([128, 1, V], F16, tag=f"st{(u * 2 + tt) % NST}", name="stage")
                    nc.vector.tensor_add(stage[:, 0, :512], psA[:], bias_rep[:, :512])
                    nc.vector.tensor_add(stage[:, 0, 512:], psB[:], bias_rep[:, 512:])
                    q = nc.sync if di % 2 == 0 else nc.gpsimd
                    di += 1
                    q.dma_start(
                        out=out[tt * 128 : (tt + 1) * 128, u : u + 1, :],
                        in_=stage[:],
                    )
            else:
                # final u: preload b_out into each PSUM bank off the
                # critical path and accumulate the matmuls onto it
                # (start=False — has_written bits are still set from the
                # bank's previous group, so the PE adds to the preload).
                # The drains become pure copies, run in PARALLEL on DVE
                # (vv0 bank) + ACT (vv1 bank), halving the end-of-kernel
                # drain chain; small DMAs ride the two HWDGE queues.
                for tt in range(T // 128):
                    for vv in range(NVB):
                        ps = mm_ps.tile([128, 512], F32, tag=f"mm{gi % 8}", name="ps")
                        gi += 1
                        nc.vector.tensor_copy(
                            ps[:], bias_rep[:, vv * 512 : (vv + 1) * 512]
                        )
                        for jc in range(NJC):
                            wt, wj = w_out_m[vv][jc]
                            nc.tensor.matmul(
                                ps[:],
                                hT[jc][:, tt * 128 : (tt + 1) * 128],
                                wt[:, wj * 512 : (wj + 1) * 512],
                                start=False,
                                stop=(jc == NJC - 1),
                                skip_group_check=True,
                            )
                        stage = st_pool.tile(
                            [128, 1, 512], F16, tag=f"sh{di % NSH}", name="hstage"
                        )
                        if vv == 0:
                            nc.vector.tensor_copy(stage[:, 0, :], ps[:])
                        else:
                            nc.scalar.activation(
                                stage[:, 0, :], ps[:], ACT_F.Identity,
                                bias=0.0, scale=1.0,
                            )
                        q = nc.sync if di % 2 == 0 else nc.scalar
                        di += 1
                        q.dma_start(
                            out=out[
                                tt * 128 : (tt + 1) * 128,
                                u : u + 1,
                                vv * 512 : (vv + 1) * 512,
                            ],
                            in_=stage[:],
                        )

    nc.finalize()
    return nc


_PROGRAM = None


def _pack(a: np.ndarray, nchunk: int) -> np.ndarray:
    """[nchunk*128, W] -> [128, nchunk*W] with pk[p, c*W+x] = a[c*128+p, x]."""
    w = a.shape[1]
    return np.ascontiguousarray(
        a.reshape(nchunk, 128, w).transpose(1, 0, 2).reshape(128, nchunk * w)
    )


def _pack_jc(a: np.ndarray) -> np.ndarray:
    """[D, J] -> [128, NJC*D] with pk[p, jc*D + dc*128 + m] = a[dc*128+p, jc*128+m].

    jc-major so each 128-wide J chunk is one contiguous DMA slice; within a
    chunk, dc-major 128-col blocks are the matmul stationary slices.
    """
    # a[dc*128+p, jc*128+m] -> out[p, jc, dc, m]
    r = a.reshape(NDC, 128, NJC, 128).transpose(1, 2, 0, 3)
    return np.ascontiguousarray(r.reshape(128, NJC * D))


def _pack_vv(a: np.ndarray) -> np.ndarray:
    """[J, V] -> [128, NJC*V] with pk[p, vv*(NJC*512) + jc*512 + x] = a[jc*128+p, vv*512+x].

    vv-major so each v-half is one contiguous DMA slice; within a half,
    jc-major 512-col blocks are the matmul moving slices.
    """
    r = a.reshape(NJC, 128, NVB, 512).transpose(1, 2, 0, 3)
    return np.ascontiguousarray(r.reshape(128, NJC * V))


def _make_in_maps(enc_out, dec_out, W_enc, b_enc, W_dec, b_dec, W_out, b_out):
    import ml_dtypes

    bf16 = ml_dtypes.bfloat16
    bb = (np.asarray(b_enc, np.float32) + np.asarray(b_dec, np.float32))
    bb_pk = np.ascontiguousarray(bb.reshape(NJC, 128).T)
    w_enc_pk = _pack_jc(np.asarray(W_enc, np.float32)).astype(bf16)
    w_dec_pk = _pack_jc(np.asarray(W_dec, np.float32)).astype(bf16)
    w_out_pk = _pack_vv(np.asarray(W_out, np.float32)).astype(bf16)
    b_out_pk = np.asarray(b_out, np.float32).reshape(1, V).astype(bf16)
    enc_f = np.asarray(enc_out, np.float32)
    dec_f = np.asarray(dec_out, np.float32)

    in_maps = []
    for b in range(B):
        in_maps.append(
            {
                "encT": _pack(np.ascontiguousarray(enc_f[b, :, 0, :].T), NDC).astype(bf16),
                "decT": _pack(np.ascontiguousarray(dec_f[b, 0, :, :].T), NDC).astype(bf16),
                "w_enc": w_enc_pk,
                "w_dec": w_dec_pk,
                "bb": bb_pk,
                "w_out": w_out_pk,
                "b_out": b_out_pk,
            }
        )
    return in_maps


def kernel(enc_out, dec_out, W_enc, b_enc, W_dec, b_dec, W_out, b_out):
    global _PROGRAM
    if _PROGRAM is None:
        _PROGRAM = build_program()

    in_maps = _make_in_maps(
        enc_out, dec_out, W_enc, b_enc, W_dec, b_dec, W_out, b_out
    )
    res = run_bass_kernel_spmd(_PROGRAM, in_maps, list(range(B)))
    out16 = np.stack([res.results[b]["out"] for b in range(B)], axis=0)
    return out16.astype(np.float32)


# revision 39
# speedup vs baseline: 1.0021x; 1.0021x over previous
"""RNN-T joint network kernel for Trainium2 (Bass/Tile), 8-core data-parallel.

Problem: out[b,t,u,:] = tanh(enc[b,t]@W_enc + b_enc + dec[b,u]@W_dec + b_dec) @ W_out + b_out
Shapes: B=8, T=256, U=64, D=512, J=640, V=1024 (all fp32).

Sharding: data-parallel over batch B across the 8 NeuronCores (1 batch element
per core). Per core the dominant work is the joint matmul (T,U,J)x(J,V):
1280 bf16 matmuls of N=512 -> ~276us at the 2.4 GHz PE clock. The 64MB fp32
output DMA (~187us at 358 GB/s) and the tanh/drain engines fit underneath.

Changes vs the 304us baseline (whose head was ~24us: serial const DMAs +
cold-clock (HAM K=4/8) projections; and whose fp32 output stream
rate-matched the ~250GB/s DMA envelope, backlogging the queues):
  - fp16 output (adds ~5e-4 rel err; total 3.4e-3 vs the 2e-2 gate), host
    upcasts to fp32 during the gather. Halves output traffic.
  - const cargo in medium (~256-384KB) need-ordered chunks split across
    both HWDGE queues: each dma_start costs ~650ns serial issue time on its
    engine, each DMA's completion sem fires ~1.5-3us after its last byte,
    so medium chunks on two queues pipeline best (~370GB/s in flight).
  - a short stream of dummy matmuls on zeroed SBUF warms the PE HAM clock
    gate during the DMA head, so projections + main loop run at 2.4 GHz.
  - bias_rep built on-device with K=1 PE broadcast matmuls (was a 512KB
    SWDGE broadcast DMA).
  - split-u head: the first U_SPLIT u's emit only their vv0 groups (vv0
    w_out half lands first), the vv1 halves backfill at u in
    [U_SPLIT, 2*U_SPLIT) — PE stays busy through the w_out arrival tail.
  - last-u drains per v-half with immediate small DMAs on the two HWDGE
    queues (SWDGE has ~2us completion latency) to shrink the tail.

Per-core layout (all J-major so J is the matmul contraction partition dim):
  host:   inputs pre-transposed and pre-packed per 128-row chunk; projection
          weights jc-major, W_out vv-major (so chunk DMAs are contiguous).
  setup:  enc_projT[j,t] = W_enc^T @ encT, dec_projT likewise with
          (b_enc+b_dec) folded in via ACT bias on the drain.
  main:   per u: hT[j,t] = tanh(enc_projT[j,t] + dec_projT[j,u]) via ACT;
          per (u,tt): two 5-mm PSUM groups (one per v-half), two [128,512]
          DVE drains adding broadcast b_out into one fp16 stage, one 512KB
          DMA (4KB/partition lines) alternating sync/gpsimd.
"""

import numpy as np
from contextlib import ExitStack

from concourse import bacc, bass, tile
from concourse.bass import mybir
from concourse.bass_utils import run_bass_kernel_spmd

F32 = mybir.dt.float32
F16 = mybir.dt.float16
BF16 = mybir.dt.bfloat16
ACT_F = mybir.ActivationFunctionType

B, T, U = 8, 256, 64
D, J, V = 512, 640, 1024
NJC = J // 128   # 5 contraction chunks of the joint matmul
NDC = D // 128   # 4 contraction chunks of the projections
NVB = V // 512   # 2 v-halves (one psum bank each) per joint output tile
N_WARM = 8       # dummy matmuls to warm the HAM clock gate during DMA head
U_SPLIT = 4      # leading u's run vv0-only, their vv1 halves backfill later
NEA = 2          # w_enc jc-chunks in the first (early) DMA tile


def build_program() -> bass.Bass:
    nc = bacc.Bacc("TRN2", target_bir_lowering=False, debug=False)

    # packed layouts (see _make_in_maps): projections jc-major, w_out vv-major
    encT_d = nc.declare_dram_parameter("encT", [128, NDC * T], BF16, isOutput=False)
    decT_d = nc.declare_dram_parameter("decT", [128, NDC * U], BF16, isOutput=False)
    w_enc_d = nc.declare_dram_parameter("w_enc", [128, NJC * D], BF16, isOutput=False)
    w_dec_d = nc.declare_dram_parameter("w_dec", [128, NJC * D], BF16, isOutput=False)
    bb_d = nc.declare_dram_parameter("bb", [128, NJC], F32, isOutput=False)  # b_enc+b_dec
    w_out_d = nc.declare_dram_parameter("w_out", [128, NJC * V], BF16, isOutput=False)
    b_out_d = nc.declare_dram_parameter("b_out", [1, V], BF16, isOutput=False)
    # fp16 output (rel err ~5e-4, negligible vs the bf16 matmul error): halves
    # the 64MB output stream, which otherwise rate-matches the ~250 GB/s
    # per-core DMA envelope and backlogs the queues
    out = nc.declare_dram_parameter("out", [T, U, V], F16, isOutput=True)

    with tile.TileContext(nc) as tc, ExitStack() as ctx:
        const = ctx.enter_context(tc.tile_pool(name="const", bufs=1))

        # --- PE warmup: HAM un-throttles after ~3.4us of sustained matmul
        # activity; run dummies on zeroed SBUF while the const DMAs stream in
        # so the projections and main loop start at 2.4 GHz instead of 1.2.
        warm_sb = const.tile([128, 512], BF16, name="warm")
        nc.vector.memset(warm_sb[:], 0.0)
        ones_sb = const.tile([1, 128], BF16, name="ones")
        nc.vector.memset(ones_sb[:], 1.0)

        # --- resident constants, split across the two HWDGE queues so the
        # dec path (sync) and enc path (scalar) load in parallel ------------
        # The const load is HBM-bandwidth-walled (~265 GB/s effective), so the
        # ORDER is what matters: dec-projection cargo first (sync), enc cargo
        # on the second HWDGE queue (scalar), then w_out chunks progressively
        # (the main loop consumes vv0 first); vv1's first chunks ride the
        # otherwise-idle gpsimd SWDGE queue. w_out chunks are separate tiles
        # so each matmul releases as soon as its own 128KB slice lands.
        # Const cargo tuning (all measured): each dma_start costs ~650ns of
        # serial ISSUE time on its engine; each DMA's completion SEMAPHORE
        # fires ~1.5-3us after its last byte (worse for bigger transfers);
        # the fabric does ~370 GB/s when >=2 transfers are in flight. So:
        # medium (~256-384KB) chunks, split across BOTH HWDGE queues for
        # parallel issue, strictly need-ordered, few enough on the scalar
        # queue that its issues retire before the projection drains + tanh.
        # sync leads with the dproj cargo (decT + w_dec) — b_out/bb are only
        # consumed later (bias matmuls / projection drains) and each DMA's
        # completion sem lags its data by ~2-3us, so first-needed goes first
        decT = const.tile([128, NDC * U], BF16)
        nc.sync.dma_start(out=decT[:], in_=decT_d[:])
        encT = const.tile([128, NDC * T], BF16)
        nc.scalar.dma_start(out=encT[:], in_=encT_d[:])

        def chunked(dram, name, splits, queues):
            """Load [128, NJC*D-like] weight as jc-chunk tiles: splits is a
            list of jc-counts; returns (tiles, per-chunk jc offset map)."""
            tiles = []
            jmap = {}
            jc0 = 0
            width = dram.shape[1] // NJC
            for si, njc in enumerate(splits):
                t = const.tile([128, njc * width], BF16, name=f"{name}{si}")
                queues[si].dma_start(
                    out=t[:], in_=dram[:, jc0 * width : (jc0 + njc) * width]
                )
                for j in range(njc):
                    jmap[jc0 + j] = (t, j)
                jc0 += njc
            return jmap

        w_dec_m = chunked(w_dec_d, "wd", [2, 3], [nc.sync, nc.sync])
        w_enc_m = chunked(w_enc_d, "we", [2, 1, 2], [nc.scalar, nc.scalar, nc.scalar])
        b_out_sb = const.tile([1, V], BF16)
        nc.sync.dma_start(out=b_out_sb[:], in_=b_out_d[:])
        bbt = const.tile([128, NJC], F32)
        nc.sync.dma_start(out=bbt[:], in_=bb_d[:])

        def w_slice(jmap, jc, dc):
            t, j = jmap[jc]
            return t[:, j * D + dc * 128 : j * D + (dc + 1) * 128]

        # w_out: per v-half, two chunk tiles (jc0-2, jc3-4), vv-major layout
        half = NJC * 512
        w_out_m = []
        for vv in range(NVB):
            m = {}
            for si, (j0, njc) in enumerate([(0, 3), (3, 2)]):
                t = const.tile([128, njc * 512], BF16, name=f"wo{vv}{si}")
                nc.sync.dma_start(
                    out=t[:],
                    in_=w_out_d[:, vv * half + j0 * 512 : vv * half + (j0 + njc) * 512],
                )
                for j in range(njc):
                    m[j0 + j] = (t, j)
            w_out_m.append(m)

        bias_rep = const.tile([128, V], F32)
        enc_projT = [const.tile([128, T], F32, name=f"ep{jc}") for jc in range(NJC)]
        dec_projT = [const.tile([128, U], F32, name=f"dp{jc}") for jc in range(NJC)]

        with tc.tile_pool(name="setup_ps", bufs=1, space="PSUM") as setup_ps:
            # dummy matmuls first in program order -> head of the PE queue.
            # MUST alternate 2 banks: same-bank WAW serializes at the
            # isolated ~610ns latency and that duty cycle does NOT trigger
            # the HAM un-throttle — the whole setup then runs at 1.2 GHz.
            for i in range(N_WARM):
                wps = setup_ps.tile([128, 512], F32, tag=f"warm{i % 2}")
                nc.tensor.matmul(
                    wps[:], warm_sb[:, :128], warm_sb[:], start=True, stop=True,
                )

            # bias_rep[p, v] = b_out[v] via K=1 broadcast matmuls; two passes
            # through ONE psum bank (pass 2 is emitted after the dec
            # projections so its WAW wait never blocks them in the PE FIFO)
            bps = setup_ps.tile([128, 512], F32, tag="biasb")
            nc.tensor.matmul(
                bps[:], ones_sb[:], b_out_sb[:, :512], start=True, stop=True,
            )
            nc.vector.tensor_copy(bias_rep[:, :512], bps[:])

            # --- input projections (bf16 mms, fp32 accumulation) ------------
            for jc in range(NJC):
                ps = setup_ps.tile([128, U], F32, tag=f"dproj{jc % 2}")
                for dc in range(NDC):
                    nc.tensor.matmul(
                        ps[:],
                        w_slice(w_dec_m, jc, dc),
                        decT[:, dc * U : (dc + 1) * U],
                        start=(dc == 0),
                        stop=(dc == NDC - 1),
                    )
                # fold b_enc+b_dec into dec_projT during the PSUM->SBUF drain
                nc.scalar.activation(
                    dec_projT[jc][:], ps[:], ACT_F.Identity,
                    bias=bbt[:, jc : jc + 1], scale=1.0,
                )

            bps2 = setup_ps.tile([128, 512], F32, tag="biasb")
            nc.tensor.matmul(
                bps2[:], ones_sb[:], b_out_sb[:, 512:], start=True, stop=True,
            )
            nc.vector.tensor_copy(bias_rep[:, 512:], bps2[:])

            for jc in range(NJC):
                ps = setup_ps.tile([128, T], F32, tag=f"eproj{jc % 3}")
                for dc in range(NDC):
                    nc.tensor.matmul(
                        ps[:],
                        w_slice(w_enc_m, jc, dc),
                        encT[:, dc * T : (dc + 1) * T],
                        start=(dc == 0),
                        stop=(dc == NDC - 1),
                    )
                nc.vector.tensor_copy(enc_projT[jc][:], ps[:])
                if jc == 1:
                    # PE filler: the w_enc/w_dec tail chunks' completion
                    # sems lag their data by ~2us; keep the PE busy through
                    # that window so HAM holds 2.4 GHz for the rest of setup
                    for i in range(8):
                        wps = setup_ps.tile([128, 512], F32, tag=f"warm{i % 2}")
                        nc.tensor.matmul(
                            wps[:], warm_sb[:, :128], warm_sb[:],
                            start=True, stop=True,
                        )

        # --- main loop, one u per iteration ---------------------------------
        # Steady state: per (u,tt) two 5-mm PSUM groups (one per v-half), two
        # [128,512] DVE drains (+ broadcast b_out) into one fp16 stage, one
        # 512KB DMA (4KB/partition lines) alternating sync/gpsimd.
        # Head: the PE consumes w_out chunks ~2x faster than HBM delivers
        # them, so u<U_SPLIT emit only their vv0 groups (vv0 chunks land
        # first) and the vv1 halves backfill at u in [U_SPLIT, 2*U_SPLIT) —
        # the PE stays busy through the w_out arrival tail.
        # Tail: last u drains per v-half with immediate small DMAs on the two
        # HWDGE queues (SWDGE has ~2us completion latency).
        h_pool = ctx.enter_context(tc.tile_pool(name="h", bufs=12))
        st_pool = ctx.enter_context(tc.tile_pool(name="stage", bufs=1))
        mm_ps = ctx.enter_context(tc.tile_pool(name="mm_ps", bufs=1, space="PSUM"))
        NST = 6   # full-stage ring depth
        NSH = 6   # half-stage ring depth

        gi = 0  # psum bank ring counter
        di = 0  # output DMA queue alternation counter
        hT_hist = {}

        def mm_group(hT, tt, vv):
            nonlocal gi
            ps = mm_ps.tile([128, 512], F32, tag=f"mm{gi % 8}", name="ps")
            gi += 1
            for jc in range(NJC):
                wt, wj = w_out_m[vv][jc]
                nc.tensor.matmul(
                    ps[:],
                    hT[jc][:, tt * 128 : (tt + 1) * 128],
                    wt[:, wj * 512 : (wj + 1) * 512],
                    start=(jc == 0),
                    stop=(jc == NJC - 1),
                )
            return ps

        def half_group(hT, u, tt, vv, hwdge_only=False):
            """5-mm group + [128,512] drain + 256KB DMA for one v-half."""
            nonlocal di
            ps = mm_group(hT, tt, vv)
            stage = st_pool.tile([128, 1, 512], F16, tag=f"sh{di % NSH}", name="hstage")
            nc.vector.tensor_add(
                stage[:, 0, :], ps[:], bias_rep[:, vv * 512 : (vv + 1) * 512]
            )
            if hwdge_only:
                q = nc.sync if di % 2 == 0 else nc.scalar
            else:
                q = nc.sync if di % 2 == 0 else nc.gpsimd
            di += 1
            q.dma_start(
                out=out[tt * 128 : (tt + 1) * 128, u : u + 1, vv * 512 : (vv + 1) * 512],
                in_=stage[:],
            )

        for u in range(U):
            hT = [h_pool.tile([128, T], BF16, tag=f"h{jc}", name=f"h{jc}") for jc in range(NJC)]
            hT_hist[u] = hT
            for jc in range(NJC):
                nc.scalar.activation(
                    hT[jc][:],
                    enc_projT[jc][:],
                    ACT_F.Tanh,
                    bias=dec_projT[jc][:, u : u + 1],
                    scale=1.0,
                )
            if U_SPLIT <= u < 2 * U_SPLIT:
                # backfill the deferred vv1 halves of u - U_SPLIT
                ub = u - U_SPLIT
                for tt in range(T // 128):
                    half_group(hT_hist[ub], ub, tt, 1)
            if u < U_SPLIT:
                for tt in range(T // 128):
                    half_group(hT_hist[u], u, tt, 0)
            elif u < U - 1:
                for tt in range(T // 128):
                    psA = mm_group(hT, tt, 0)
                    psB = mm_group(hT, tt, 1)
                    stage = st_pool.tile([128, 1, V], F16, tag=f"st{(u * 2 + tt) % NST}", name="stage")
                    nc.vector.tensor_add(stage[:, 0, :512], psA[:], bias_rep[:, :512])
                    nc.vector.tensor_add(stage[:, 0, 512:], psB[:], bias_rep[:, 512:])
                    q = nc.sync if di % 2 == 0 else nc.gpsimd
                    di += 1
                    q.dma_start(
                        out=out[tt * 128 : (tt + 1) * 128, u : u + 1, :],
                        in_=stage[:],
                    )
            else:
                for tt in range(T // 128):
                    for vv in range(NVB):
                        half_group(hT, u, tt, vv, hwdge_only=True)

    nc.finalize()
    return nc


_PROGRAM = None


def _pack(a: np.ndarray, nchunk: int) -> np.ndarray:
    """[nchunk*128, W] -> [128, nchunk*W] with pk[p, c*W+x] = a[c*128+p, x]."""
    w = a.shape[1]
    return np.ascontiguousarray(
        a.reshape(nchunk, 128, w).transpose(1, 0, 2).reshape(128, nchunk * w)
    )


def _pack_jc(a: np.ndarray) -> np.ndarray:
    """[D, J] -> [128, NJC*D] with pk[p, jc*D + dc*128 + m] = a[dc*128+p, jc*128+m].

    jc-major so each 128-wide J chunk is one contiguous DMA slice; within a
    chunk, dc-major 128-col blocks are the matmul stationary slices.
    """
    # a[dc*128+p, jc*128+m] -> out[p, jc, dc, m]
    r = a.reshape(NDC, 128, NJC, 128).transpose(1, 2, 0, 3)
    return np.ascontiguousarray(r.reshape(128, NJC * D))


def _pack_vv(a: np.ndarray) -> np.ndarray:
    """[J, V] -> [128, NJC*V] with pk[p, vv*(NJC*512) + jc*512 + x] = a[jc*128+p, vv*512+x].

    vv-major so each v-half is one contiguous DMA slice; within a half,
    jc-major 512-col blocks are the matmul moving slices.
    """
    r = a.reshape(NJC, 128, NVB, 512).transpose(1, 2, 0, 3)
    return np.ascontiguousarray(r.reshape(128, NJC * V))


def _make_in_maps(enc_out, dec_out, W_enc, b_enc, W_dec, b_dec, W_out, b_out):
    import ml_dtypes

    bf16 = ml_dtypes.bfloat16
    bb = (np.asarray(b_enc, np.float32) + np.asarray(b_dec, np.float32))
    bb_pk = np.ascontiguousarray(bb.reshape(NJC, 128).T)
    w_enc_pk = _pack_jc(np.asarray(W_enc, np.float32)).astype(bf16)
    w_dec_pk = _pack_jc(np.asarray(W_dec, np.float32)).astype(bf16)
    w_out_pk = _pack_vv(np.asarray(W_out, np.float32)).astype(bf16)
    b_out_pk = np.asarray(b_out, np.float32).reshape(1, V).astype(bf16)
    enc_f = np.asarray(enc_out, np.float32)
    dec_f = np.asarray(dec_out, np.float32)

    in_maps = []
    for b in range(B):
        in_maps.append(
            {
                "encT": _pack(np.ascontiguousarray(enc_f[b, :, 0, :].T), NDC).astype(bf16),
                "decT": _pack(np.ascontiguousarray(dec_f[b, 0, :, :].T), NDC).astype(bf16),
                "w_enc": w_enc_pk,
                "w_dec": w_dec_pk,
                "bb": bb_pk,
                "w_out": w_out_pk,
                "b_out": b_out_pk,
            }
        )
    return in_maps


def kernel(enc_out, dec_out, W_enc, b_enc, W_dec, b_dec, W_out, b_out):
    global _PROGRAM
    if _PROGRAM is None:
        _PROGRAM = build_program()

    in_maps = _make_in_maps(
        enc_out, dec_out, W_enc, b_enc, W_dec, b_dec, W_out, b_out
    )
    res = run_bass_kernel_spmd(_PROGRAM, in_maps, list(range(B)))
    out16 = np.stack([res.results[b]["out"] for b in range(B)], axis=0)
    return out16.astype(np.float32)


# revision 40
# speedup vs baseline: 1.0038x; 1.0018x over previous
"""RNN-T joint network kernel for Trainium2 (Bass/Tile), 8-core data-parallel.

Problem: out[b,t,u,:] = tanh(enc[b,t]@W_enc + b_enc + dec[b,u]@W_dec + b_dec) @ W_out + b_out
Shapes: B=8, T=256, U=64, D=512, J=640, V=1024 (all fp32).

Sharding: data-parallel over batch B across the 8 NeuronCores (1 batch element
per core). Per core the dominant work is the joint matmul (T,U,J)x(J,V):
1280 bf16 matmuls of N=512 -> ~276us at the 2.4 GHz PE clock. The 64MB fp32
output DMA (~187us at 358 GB/s) and the tanh/drain engines fit underneath.

Changes vs the 304us baseline (whose head was ~24us: serial const DMAs +
cold-clock (HAM K=4/8) projections; and whose fp32 output stream
rate-matched the ~250GB/s DMA envelope, backlogging the queues):
  - fp16 output (adds ~5e-4 rel err; total 3.4e-3 vs the 2e-2 gate), host
    upcasts to fp32 during the gather. Halves output traffic.
  - const cargo in medium (~256-384KB) need-ordered chunks split across
    both HWDGE queues: each dma_start costs ~650ns serial issue time on its
    engine, each DMA's completion sem fires ~1.5-3us after its last byte,
    so medium chunks on two queues pipeline best (~370GB/s in flight).
  - a short stream of dummy matmuls on zeroed SBUF warms the PE HAM clock
    gate during the DMA head, so projections + main loop run at 2.4 GHz.
  - bias_rep built on-device with K=1 PE broadcast matmuls (was a 512KB
    SWDGE broadcast DMA).
  - split-u head: the first U_SPLIT u's emit only their vv0 groups (vv0
    w_out half lands first), the vv1 halves backfill at u in
    [U_SPLIT, 2*U_SPLIT) — PE stays busy through the w_out arrival tail.
  - last-u drains per v-half with immediate small DMAs on the two HWDGE
    queues (SWDGE has ~2us completion latency) to shrink the tail.

Per-core layout (all J-major so J is the matmul contraction partition dim):
  host:   inputs pre-transposed and pre-packed per 128-row chunk; projection
          weights jc-major, W_out vv-major (so chunk DMAs are contiguous).
  setup:  enc_projT[j,t] = W_enc^T @ encT, dec_projT likewise with
          (b_enc+b_dec) folded in via ACT bias on the drain.
  main:   per u: hT[j,t] = tanh(enc_projT[j,t] + dec_projT[j,u]) via ACT;
          per (u,tt): two 5-mm PSUM groups (one per v-half), two [128,512]
          DVE drains adding broadcast b_out into one fp16 stage, one 512KB
          DMA (4KB/partition lines) alternating sync/gpsimd.
"""

import numpy as np
from contextlib import ExitStack

from concourse import bacc, bass, tile
from concourse.bass import mybir
from concourse.bass_utils import run_bass_kernel_spmd

F32 = mybir.dt.float32
F16 = mybir.dt.float16
BF16 = mybir.dt.bfloat16
ACT_F = mybir.ActivationFunctionType

B, T, U = 8, 256, 64
D, J, V = 512, 640, 1024
NJC = J // 128   # 5 contraction chunks of the joint matmul
NDC = D // 128   # 4 contraction chunks of the projections
NVB = V // 512   # 2 v-halves (one psum bank each) per joint output tile
N_WARM = 8       # dummy matmuls to warm the HAM clock gate during DMA head
U_SPLIT = 4      # leading u's run vv0-only, their vv1 halves backfill later
NEA = 2          # w_enc jc-chunks in the first (early) DMA tile


def build_program() -> bass.Bass:
    nc = bacc.Bacc("TRN2", target_bir_lowering=False, debug=False)

    # packed layouts (see _make_in_maps): projections jc-major, w_out vv-major
    encT_d = nc.declare_dram_parameter("encT", [128, NDC * T], BF16, isOutput=False)
    decT_d = nc.declare_dram_parameter("decT", [128, NDC * U], BF16, isOutput=False)
    w_enc_d = nc.declare_dram_parameter("w_enc", [128, NJC * D], BF16, isOutput=False)
    w_dec_d = nc.declare_dram_parameter("w_dec", [128, NJC * D], BF16, isOutput=False)
    bb_d = nc.declare_dram_parameter("bb", [128, NJC], F32, isOutput=False)  # b_enc+b_dec
    w_out_d = nc.declare_dram_parameter("w_out", [128, NJC * V], BF16, isOutput=False)
    b_out_d = nc.declare_dram_parameter("b_out", [1, V], BF16, isOutput=False)
    # fp16 output (rel err ~5e-4, negligible vs the bf16 matmul error): halves
    # the 64MB output stream, which otherwise rate-matches the ~250 GB/s
    # per-core DMA envelope and backlogs the queues
    out = nc.declare_dram_parameter("out", [T, U, V], F16, isOutput=True)

    with tile.TileContext(nc) as tc, ExitStack() as ctx:
        const = ctx.enter_context(tc.tile_pool(name="const", bufs=1))

        # --- PE warmup: HAM un-throttles after ~3.4us of sustained matmul
        # activity; run dummies on zeroed SBUF while the const DMAs stream in
        # so the projections and main loop start at 2.4 GHz instead of 1.2.
        warm_sb = const.tile([128, 512], BF16, name="warm")
        nc.vector.memset(warm_sb[:], 0.0)
        ones_sb = const.tile([1, 128], BF16, name="ones")
        nc.vector.memset(ones_sb[:], 1.0)

        # --- resident constants, split across the two HWDGE queues so the
        # dec path (sync) and enc path (scalar) load in parallel ------------
        # The const load is HBM-bandwidth-walled (~265 GB/s effective), so the
        # ORDER is what matters: dec-projection cargo first (sync), enc cargo
        # on the second HWDGE queue (scalar), then w_out chunks progressively
        # (the main loop consumes vv0 first); vv1's first chunks ride the
        # otherwise-idle gpsimd SWDGE queue. w_out chunks are separate tiles
        # so each matmul releases as soon as its own 128KB slice lands.
        # Const cargo tuning (all measured): each dma_start costs ~650ns of
        # serial ISSUE time on its engine; each DMA's completion SEMAPHORE
        # fires ~1.5-3us after its last byte (worse for bigger transfers);
        # the fabric does ~370 GB/s when >=2 transfers are in flight. So:
        # medium (~256-384KB) chunks, split across BOTH HWDGE queues for
        # parallel issue, strictly need-ordered, few enough on the scalar
        # queue that its issues retire before the projection drains + tanh.
        # sync leads with the dproj cargo (decT + w_dec) — b_out/bb are only
        # consumed later (bias matmuls / projection drains) and each DMA's
        # completion sem lags its data by ~2-3us, so first-needed goes first
        decT = const.tile([128, NDC * U], BF16)
        nc.sync.dma_start(out=decT[:], in_=decT_d[:])
        encT = const.tile([128, NDC * T], BF16)
        nc.scalar.dma_start(out=encT[:], in_=encT_d[:])

        def chunked(dram, name, splits, queues):
            """Load [128, NJC*D-like] weight as jc-chunk tiles: splits is a
            list of jc-counts; returns (tiles, per-chunk jc offset map)."""
            tiles = []
            jmap = {}
            jc0 = 0
            width = dram.shape[1] // NJC
            for si, njc in enumerate(splits):
                t = const.tile([128, njc * width], BF16, name=f"{name}{si}")
                queues[si].dma_start(
                    out=t[:], in_=dram[:, jc0 * width : (jc0 + njc) * width]
                )
                for j in range(njc):
                    jmap[jc0 + j] = (t, j)
                jc0 += njc
            return jmap

        w_dec_m = chunked(w_dec_d, "wd", [2, 3], [nc.sync, nc.sync])
        w_enc_m = chunked(w_enc_d, "we", [2, 1, 2], [nc.scalar, nc.scalar, nc.scalar])
        b_out_sb = const.tile([1, V], BF16)
        nc.sync.dma_start(out=b_out_sb[:], in_=b_out_d[:])
        bbt = const.tile([128, NJC], F32)
        nc.sync.dma_start(out=bbt[:], in_=bb_d[:])

        def w_slice(jmap, jc, dc):
            t, j = jmap[jc]
            return t[:, j * D + dc * 128 : j * D + (dc + 1) * 128]

        # w_out: per v-half, two chunk tiles (jc0-2, jc3-4), vv-major layout
        half = NJC * 512
        w_out_m = []
        for vv in range(NVB):
            m = {}
            for si, (j0, njc) in enumerate([(0, 3), (3, 2)]):
                t = const.tile([128, njc * 512], BF16, name=f"wo{vv}{si}")
                nc.sync.dma_start(
                    out=t[:],
                    in_=w_out_d[:, vv * half + j0 * 512 : vv * half + (j0 + njc) * 512],
                )
                for j in range(njc):
                    m[j0 + j] = (t, j)
            w_out_m.append(m)

        bias_rep = const.tile([128, V], F32)
        enc_projT = [const.tile([128, T], F32, name=f"ep{jc}") for jc in range(NJC)]
        dec_projT = [const.tile([128, U], F32, name=f"dp{jc}") for jc in range(NJC)]

        with tc.tile_pool(name="setup_ps", bufs=1, space="PSUM") as setup_ps:
            # dummy matmuls first in program order -> head of the PE queue.
            # MUST alternate 2 banks: same-bank WAW serializes at the
            # isolated ~610ns latency and that duty cycle does NOT trigger
            # the HAM un-throttle — the whole setup then runs at 1.2 GHz.
            for i in range(N_WARM):
                wps = setup_ps.tile([128, 512], F32, tag=f"warm{i % 2}")
                nc.tensor.matmul(
                    wps[:], warm_sb[:, :128], warm_sb[:], start=True, stop=True,
                )

            # bias_rep[p, v] = b_out[v] via K=1 broadcast matmuls; two passes
            # through ONE psum bank (pass 2 is emitted after the dec
            # projections so its WAW wait never blocks them in the PE FIFO)
            bps = setup_ps.tile([128, 512], F32, tag="biasb")
            nc.tensor.matmul(
                bps[:], ones_sb[:], b_out_sb[:, :512], start=True, stop=True,
            )
            nc.vector.tensor_copy(bias_rep[:, :512], bps[:])

            # --- input projections (bf16 mms, fp32 accumulation) ------------
            for jc in range(NJC):
                ps = setup_ps.tile([128, U], F32, tag=f"dproj{jc % 2}")
                for dc in range(NDC):
                    nc.tensor.matmul(
                        ps[:],
                        w_slice(w_dec_m, jc, dc),
                        decT[:, dc * U : (dc + 1) * U],
                        start=(dc == 0),
                        stop=(dc == NDC - 1),
                    )
                # fold b_enc+b_dec into dec_projT during the PSUM->SBUF drain
                nc.scalar.activation(
                    dec_projT[jc][:], ps[:], ACT_F.Identity,
                    bias=bbt[:, jc : jc + 1], scale=1.0,
                )

            bps2 = setup_ps.tile([128, 512], F32, tag="biasb")
            nc.tensor.matmul(
                bps2[:], ones_sb[:], b_out_sb[:, 512:], start=True, stop=True,
            )
            nc.vector.tensor_copy(bias_rep[:, 512:], bps2[:])

            for jc in range(NJC):
                ps = setup_ps.tile([128, T], F32, tag=f"eproj{jc % 3}")
                for dc in range(NDC):
                    nc.tensor.matmul(
                        ps[:],
                        w_slice(w_enc_m, jc, dc),
                        encT[:, dc * T : (dc + 1) * T],
                        start=(dc == 0),
                        stop=(dc == NDC - 1),
                    )
                nc.vector.tensor_copy(enc_projT[jc][:], ps[:])

        # --- main loop, one u per iteration ---------------------------------
        # Steady state: per (u,tt) two 5-mm PSUM groups (one per v-half), two
        # [128,512] DVE drains (+ broadcast b_out) into one fp16 stage, one
        # 512KB DMA (4KB/partition lines) alternating sync/gpsimd.
        # Head: the PE consumes w_out chunks ~2x faster than HBM delivers
        # them, so u<U_SPLIT emit only their vv0 groups (vv0 chunks land
        # first) and the vv1 halves backfill at u in [U_SPLIT, 2*U_SPLIT) —
        # the PE stays busy through the w_out arrival tail.
        # Tail: last u drains per v-half with immediate small DMAs on the two
        # HWDGE queues (SWDGE has ~2us completion latency).
        h_pool = ctx.enter_context(tc.tile_pool(name="h", bufs=12))
        st_pool = ctx.enter_context(tc.tile_pool(name="stage", bufs=1))
        mm_ps = ctx.enter_context(tc.tile_pool(name="mm_ps", bufs=1, space="PSUM"))
        NST = 6   # full-stage ring depth
        NSH = 6   # half-stage ring depth

        gi = 0  # psum bank ring counter
        di = 0  # output DMA queue alternation counter
        hT_hist = {}

        def mm_group(hT, tt, vv):
            nonlocal gi
            ps = mm_ps.tile([128, 512], F32, tag=f"mm{gi % 8}", name="ps")
            gi += 1
            for jc in range(NJC):
                wt, wj = w_out_m[vv][jc]
                nc.tensor.matmul(
                    ps[:],
                    hT[jc][:, tt * 128 : (tt + 1) * 128],
                    wt[:, wj * 512 : (wj + 1) * 512],
                    start=(jc == 0),
                    stop=(jc == NJC - 1),
                )
            return ps

        def half_group(hT, u, tt, vv, hwdge_only=False):
            """5-mm group + [128,512] drain + 256KB DMA for one v-half."""
            nonlocal di
            ps = mm_group(hT, tt, vv)
            stage = st_pool.tile([128, 1, 512], F16, tag=f"sh{di % NSH}", name="hstage")
            nc.vector.tensor_add(
                stage[:, 0, :], ps[:], bias_rep[:, vv * 512 : (vv + 1) * 512]
            )
            if hwdge_only:
                q = nc.sync if di % 2 == 0 else nc.scalar
            else:
                q = nc.sync if di % 2 == 0 else nc.gpsimd
            di += 1
            q.dma_start(
                out=out[tt * 128 : (tt + 1) * 128, u : u + 1, vv * 512 : (vv + 1) * 512],
                in_=stage[:],
            )

        for u in range(U):
            hT = [h_pool.tile([128, T], BF16, tag=f"h{jc}", name=f"h{jc}") for jc in range(NJC)]
            hT_hist[u] = hT
            for jc in range(NJC):
                nc.scalar.activation(
                    hT[jc][:],
                    enc_projT[jc][:],
                    ACT_F.Tanh,
                    bias=dec_projT[jc][:, u : u + 1],
                    scale=1.0,
                )
            if U_SPLIT <= u < 2 * U_SPLIT:
                # backfill the deferred vv1 halves of u - U_SPLIT
                ub = u - U_SPLIT
                for tt in range(T // 128):
                    half_group(hT_hist[ub], ub, tt, 1)
            if u < U_SPLIT:
                for tt in range(T // 128):
                    half_group(hT_hist[u], u, tt, 0)
            elif u < U - 1:
                for tt in range(T // 128):
                    psA = mm_group(hT, tt, 0)
                    psB = mm_group(hT, tt, 1)
                    stage = st_pool.tile([128, 1, V], F16, tag=f"st{(u * 2 + tt) % NST}", name="stage")
                    nc.vector.tensor_add(stage[:, 0, :512], psA[:], bias_rep[:, :512])
                    nc.vector.tensor_add(stage[:, 0, 512:], psB[:], bias_rep[:, 512:])
                    q = nc.sync if di % 2 == 0 else nc.gpsimd
                    di += 1
                    q.dma_start(
                        out=out[tt * 128 : (tt + 1) * 128, u : u + 1, :],
                        in_=stage[:],
                    )
            else:
                for tt in range(T // 128):
                    for vv in range(NVB):
                        half_group(hT, u, tt, vv, hwdge_only=True)

    nc.finalize()
    return nc


_PROGRAM = None


def _pack(a: np.ndarray, nchunk: int) -> np.ndarray:
    """[nchunk*128, W] -> [128, nchunk*W] with pk[p, c*W+x] = a[c*128+p, x]."""
    w = a.shape[1]
    return np.ascontiguousarray(
        a.reshape(nchunk, 128, w).transpose(1, 0, 2).reshape(128, nchunk * w)
    )


def _pack_jc(a: np.ndarray) -> np.ndarray:
    """[D, J] -> [128, NJC*D] with pk[p, jc*D + dc*128 + m] = a[dc*128+p, jc*128+m].

    jc-major so each 128-wide J chunk is one contiguous DMA slice; within a
    chunk, dc-major 128-col blocks are the matmul stationary slices.
    """
    # a[dc*128+p, jc*128+m] -> out[p, jc, dc, m]
    r = a.reshape(NDC, 128, NJC, 128).transpose(1, 2, 0, 3)
    return np.ascontiguousarray(r.reshape(128, NJC * D))


def _pack_vv(a: np.ndarray) -> np.ndarray:
    """[J, V] -> [128, NJC*V] with pk[p, vv*(NJC*512) + jc*512 + x] = a[jc*128+p, vv*512+x].

    vv-major so each v-half is one contiguous DMA slice; within a half,
    jc-major 512-col blocks are the matmul moving slices.
    """
    r = a.reshape(NJC, 128, NVB, 512).transpose(1, 2, 0, 3)
    return np.ascontiguousarray(r.reshape(128, NJC * V))


def _make_in_maps(enc_out, dec_out, W_enc, b_enc, W_dec, b_dec, W_out, b_out):
    import ml_dtypes

    bf16 = ml_dtypes.bfloat16
    bb = (np.asarray(b_enc, np.float32) + np.asarray(b_dec, np.float32))
    bb_pk = np.ascontiguousarray(bb.reshape(NJC, 128).T)
    w_enc_pk = _pack_jc(np.asarray(W_enc, np.float32)).astype(bf16)
    w_dec_pk = _pack_jc(np.asarray(W_dec, np.float32)).astype(bf16)
    w_out_pk = _pack_vv(np.asarray(W_out, np.float32)).astype(bf16)
    b_out_pk = np.asarray(b_out, np.float32).reshape(1, V).astype(bf16)
    enc_f = np.asarray(enc_out, np.float32)
    dec_f = np.asarray(dec_out, np.float32)

    in_maps = []
    for b in range(B):
        in_maps.append(
            {
                "encT": _pack(np.ascontiguousarray(enc_f[b, :, 0, :].T), NDC).astype(bf16),
                "decT": _pack(np.ascontiguousarray(dec_f[b, 0, :, :].T), NDC).astype(bf16),
                "w_enc": w_enc_pk,
                "w_dec": w_dec_pk,
                "bb": bb_pk,
                "w_out": w_out_pk,
                "b_out": b_out_pk,
            }
        )
    return in_maps


def kernel(enc_out, dec_out, W_enc, b_enc, W_dec, b_dec, W_out, b_out):
    global _PROGRAM
    if _PROGRAM is None:
        _PROGRAM = build_program()

    in_maps = _make_in_maps(
        enc_out, dec_out, W_enc, b_enc, W_dec, b_dec, W_out, b_out
    )
    res = run_bass_kernel_spmd(_PROGRAM, in_maps, list(range(B)))
    out16 = np.stack([res.results[b]["out"] for b in range(B)], axis=0)
    return out16.astype(np.float32)
